# revision 2
# baseline (speedup 1.0000x reference)
"""DeepseekV2 MLA attention (B=1, S=2048, H=4096, NH=32) on 8 TRN2 cores.

Sharding: tensor-parallel over heads (4 heads/core).  The q_a projection +
RMSNorm runs data-parallel over sequence (each core does its 256-row slice)
and is AllGathered; the (cheaper) kv_a front is replicated per core so the
K/V projections can proceed while the AllGather is in flight.  Each core
emits a partial output projection (its head slice of Wo); the host sums the
8 partials.

All matmul operands are pre-transposed/packed on the HOST into T-layout
([feature, seq]) so the PE always contracts over the partition dim with zero
on-device transposes.  RMSNorm ln weights and the softmax scale are folded
into Wqb/Wkvb host-side.  Attention runs as logits^T [k, q]: softmax over
the partition axis via ones-matmul denominators, no max subtraction (logits
are O(5) for randn inputs), mask applied as data (causal tiles skipped only
when the host verifies the mask is exactly causal).

Matmuls run in float32r (full-rate PE; ~3e-4 rel err end to end).
"""

import ctypes
import os
import numpy as np

import concourse.bass as bass
import concourse.mybir as mybir
from concourse.tile import TileContext
import concourse.bass_utils as bass_utils
from concourse.bass_utils import run_bass_kernel_spmd

bass_utils.upload_artifacts = lambda tmpdir: tmpdir  # no artifact bucket here

S = 2048
H = 4096
NCORES = 8
NHC = 4            # heads per core
NOPE, ROPE, VD = 128, 64, 128
QHD = NOPE + ROPE  # 192
QLR, KVLR = 1536, 512
BASE = 10000.0
EPS = 1e-6
SCALE = QHD ** -0.5
P = 128
SC = 512           # seq chunk (local phases)
SLC = S // NCORES  # 256, per-core front slice
NSC = S // SC      # 4
NKB = S // P       # 16 key blocks
FR = mybir.dt.float32r
F32 = mybir.dt.float32
AF = mybir.ActivationFunctionType

N_KI = H // P      # 32 front contraction tiles
NQB = QLR // P     # 12
NKVB = KVLR // P   # 4
FB_W = [P] * NQB + [P] * NKVB + [ROPE]  # 17 front output blocks
N_FB = len(FB_W)


def axon_reset():
    import jax
    jax.devices()
    lib = ctypes.CDLL('/opt/axon/libaxon_pjrt.so')
    lib.axon_reset.restype = ctypes.c_int64
    return lib.axon_reset()


def split_multiwaits(nc, cap=1):
    """This walrus pin allows only `cap` sync-waits per instruction; spill
    extras onto same-engine NoOps inserted just before the instruction."""
    for f in nc.m.functions:
        for b in f.blocks:
            li = b.instructions
            out = []
            changed = False
            for inst in list(li):
                si = getattr(inst, "sync_info", None)
                waits = list(si.on_wait) if si is not None and si.on_wait else []
                if len(waits) > cap:
                    changed = True
                    extra, keep = waits[:-cap], waits[-cap:]
                    for j in range(0, len(extra), cap):
                        out.append(mybir.InstNoOp(
                            name=nc.get_next_instruction_name(),
                            engine=inst.engine, ins=[], outs=[],
                            sync_info=mybir.SyncInfo(
                                on_wait=extra[j:j + cap], on_update=[]),
                            bass_nofuse=True,
                        ))
                    inst.sync_info = mybir.SyncInfo(
                        on_wait=keep, on_update=list(si.on_update))
                out.append(inst)
            if changed:
                li[:] = out


def build(causal: bool) -> bass.Bass:
    nc = bass.Bass()
    hT = nc.declare_dram_parameter("hT", [H, S], F32, isOutput=False)
    hTs = nc.declare_dram_parameter("hTs", [H, SLC], F32, isOutput=False)
    maskT = nc.declare_dram_parameter("maskT", [S, S], F32, isOutput=False)
    Wp = nc.declare_dram_parameter("Wp", [P, N_FB * N_KI * P], F32, isOutput=False)
    Wqb_p = nc.declare_dram_parameter("Wqb_p", [P, NQB * NHC * QHD], F32, isOutput=False)
    Wkvb_p = nc.declare_dram_parameter("Wkvb_p", [P, NKVB * NHC * (NOPE + VD)], F32, isOutput=False)
    Wo_p = nc.declare_dram_parameter("Wo_p", [P, NKVB * H], F32, isOutput=False)
    cq = nc.declare_dram_parameter("cq", [ROPE, S], F32, isOutput=False)
    sq = nc.declare_dram_parameter("sq", [ROPE, S], F32, isOutput=False)
    outT = nc.declare_dram_parameter("outT", [H, S], F32, isOutput=True)

    Wp3 = Wp.rearrange("p (fk w) -> p fk w", w=P)        # [P, 17*32, 128]
    Wqb3 = Wqb_p.rearrange("p (k w) -> p k w", k=NQB)    # [P, 12, 768]
    Wkvb3 = Wkvb_p.rearrange("p (k w) -> p k w", k=NKVB)  # [P, 4, 1024]
    Wo3 = Wo_p.rearrange("p (k w) -> p k w", k=NKVB)     # [P, 4, 4096]

    def fr(ap):
        return ap.bitcast(FR)

    with TileContext(nc) as tc:
        with (
            tc.tile_pool(name="dram", bufs=1, space="DRAM") as dpool,
            tc.tile_pool(name="const", bufs=1) as cpool,
        ):
            kvnT = dpool.tile([KVLR, S], F32)
            qnT = dpool.tile([NHC * NOPE, S], F32)
            qrT = dpool.tile([NHC * ROPE, S], F32)
            kpeT = dpool.tile([ROPE, S], F32)
            onT = dpool.tile([NHC * VD, S], F32)
            cc_q_in = dpool.tile([QLR, SLC], F32)
            cc_q_out = dpool.tile([NCORES, QLR, SLC], F32, addr_space="Shared")
            ones_f = cpool.tile([P, 1], F32)
            nc.vector.memset(ones_f[:], 1.0)
            ones_rf = cpool.tile([1, P], F32)
            nc.vector.memset(ones_rf[:], 1.0)
            ones_t = cpool.tile([P, 1], FR)
            nc.scalar.copy(ones_t[:], ones_f[:])
            ones_row = cpool.tile([1, P], FR)
            nc.scalar.copy(ones_row[:], ones_rf[:])

            # ------------- Phase 1: front projections + RMSNorm + k rope
            with (
                tc.tile_pool(name="hcol", bufs=1) as hpool,
                tc.tile_pool(name="wfr", bufs=2) as wpool,
                tc.tile_pool(name="raw", bufs=1) as rpool,
                tc.tile_pool(name="nrm", bufs=2) as npool,
                tc.tile_pool(name="ckr", bufs=1) as ckpool,
                tc.tile_pool(name="ps", bufs=3, space="PSUM") as pspool,
                tc.tile_pool(name="ps1", bufs=1, space="PSUM") as ps1pool,
            ):
                # --- 1q: q_a on the local 256-col slice, then AllGather
                hqs = []
                for ki in range(N_KI):
                    ht = hpool.tile([P, SLC], FR, tag=f"h{ki}", name=f"hq{ki}")
                    nc.gpsimd.dma_start(out=ht[:], in_=hTs[ki * P:(ki + 1) * P, :])
                    hqs.append(ht)
                qraws = []
                sq_qp = ps1pool.tile([1, SLC], F32, tag="sq_q")
                for fb in range(NQB):
                    wt = wpool.tile([P, N_KI, P], FR, tag="w", name=f"wq{fb}")
                    nc.gpsimd.dma_start(
                        out=wt[:], in_=Wp3[:, fb * N_KI:(fb + 1) * N_KI, :])
                    ps = pspool.tile([P, SLC], F32, tag="ps", name=f"psq{fb}")
                    for ki in range(N_KI):
                        nc.tensor.matmul(ps[:], lhsT=fr(wt[:, ki, :]), rhs=hqs[ki][:],
                                         start=(ki == 0), stop=(ki == N_KI - 1))
                    raw = rpool.tile([P, SLC], F32, tag=f"r{fb}", name=f"rq{fb}")
                    nc.scalar.copy(raw[:], ps[:])
                    qraws.append(raw)
                    sqt = npool.tile([P, SLC], FR, tag="sqt", name=f"sqtq{fb}")
                    nc.vector.tensor_mul(sqt[:], raw[:], raw[:])
                    nc.tensor.matmul(sq_qp[:], lhsT=ones_t[:], rhs=sqt[:],
                                     start=(fb == 0), stop=(fb == NQB - 1))
                ms = npool.tile([1, SLC], F32, tag="ms", name="msq")
                nc.scalar.activation(ms[:], sq_qp[:], AF.Copy, scale=1.0 / QLR, bias=EPS)
                rc = npool.tile([1, SLC], F32, tag="rc", name="rcq")
                nc.vector.reciprocal(rc[:], ms[:])
                rs = npool.tile([1, SLC], FR, tag="rs", name="rsq")
                nc.scalar.activation(rs[:], rc[:], AF.Sqrt)
                bps = ps1pool.tile([P, SLC], F32, tag="bps", name="bpsq")
                nc.tensor.matmul(bps[:], lhsT=ones_row[:], rhs=rs[:], start=True, stop=True)
                rb = npool.tile([P, SLC], F32, tag="rb", name="rbq")
                nc.scalar.copy(rb[:], bps[:])
                for j in range(NQB):
                    nt = npool.tile([P, SLC], F32, tag="nt", name=f"ntq{j}")
                    nc.vector.tensor_mul(nt[:], qraws[j][:], rb[:])
                    nc.gpsimd.dma_start(out=cc_q_in[j * P:(j + 1) * P, :], in_=nt[:])
                nc.gpsimd.collective_compute(
                    "AllGather", mybir.AluOpType.bypass,
                    replica_groups=[list(range(NCORES))],
                    ins=[cc_q_in.opt()], outs=[cc_q_out.opt()])

                # --- 1kv: kv_a + rope over the full sequence (replicated)
                ck_t = ckpool.tile([ROPE, S], F32, tag="ck")
                sk_t = ckpool.tile([ROPE, S], F32, tag="sk")
                nc.gpsimd.dma_start(out=ck_t[:], in_=cq[:, :])
                nc.gpsimd.dma_start(out=sk_t[:], in_=sq[:, :])
                for sc in range(NSC):
                    ssl = slice(sc * SC, (sc + 1) * SC)
                    hts = []
                    for ki in range(N_KI):
                        ht = hpool.tile([P, SC], FR, tag=f"h{ki}", name=f"hk{ki}_{sc}")
                        nc.gpsimd.dma_start(out=ht[:], in_=hT[ki * P:(ki + 1) * P, ssl])
                        hts.append(ht)
                    raws = []
                    sq_kv = ps1pool.tile([1, SC], F32, tag="sq_kv")
                    for fbi, fb in enumerate(range(NQB, N_FB)):
                        w = FB_W[fb]
                        wt = wpool.tile([P, N_KI, P], FR, tag="w", name=f"wk{fb}_{sc}")
                        nc.gpsimd.dma_start(
                            out=wt[:], in_=Wp3[:, fb * N_KI:(fb + 1) * N_KI, :])
                        ps = pspool.tile([P, SC], F32, tag="ps", name=f"psk{fb}_{sc}")
                        for ki in range(N_KI):
                            nc.tensor.matmul(ps[:w, :], lhsT=fr(wt[:, ki, :w]), rhs=hts[ki][:],
                                             start=(ki == 0), stop=(ki == N_KI - 1))
                        raw = rpool.tile([P, SC], F32, tag=f"r{fb}", name=f"rk{fb}_{sc}")
                        nc.scalar.copy(raw[:w, :], ps[:w, :])
                        raws.append(raw)
                        if fb < NQB + NKVB:
                            sqt = npool.tile([P, SC], FR, tag="sqt", name=f"sqtk{fb}_{sc}")
                            nc.vector.tensor_mul(sqt[:], raw[:], raw[:])
                            nc.tensor.matmul(sq_kv[:], lhsT=ones_t[:], rhs=sqt[:],
                                             start=(fb == NQB), stop=(fb == NQB + NKVB - 1))
                    ms = npool.tile([1, SC], F32, tag="ms", name=f"msk{sc}")
                    nc.scalar.activation(ms[:], sq_kv[:], AF.Copy, scale=1.0 / KVLR, bias=EPS)
                    rc = npool.tile([1, SC], F32, tag="rc", name=f"rck{sc}")
                    nc.vector.reciprocal(rc[:], ms[:])
                    rs = npool.tile([1, SC], FR, tag="rs", name=f"rsk{sc}")
                    nc.scalar.activation(rs[:], rc[:], AF.Sqrt)
                    bps = ps1pool.tile([P, SC], F32, tag="bps", name=f"bpsk{sc}")
                    nc.tensor.matmul(bps[:], lhsT=ones_row[:], rhs=rs[:], start=True, stop=True)
                    rb = npool.tile([P, SC], F32, tag="rb", name=f"rbk{sc}")
                    nc.scalar.copy(rb[:], bps[:])
                    for j in range(NKVB):
                        nt = npool.tile([P, SC], F32, tag="nt", name=f"ntk{j}_{sc}")
                        nc.vector.tensor_mul(nt[:], raws[j][:], rb[:])
                        nc.gpsimd.dma_start(out=kvnT[j * P:(j + 1) * P, ssl], in_=nt[:])
                    kraw = raws[NKVB]
                    ksw = npool.tile([ROPE, SC], F32, tag="ksw", name=f"ksw{sc}")
                    nc.gpsimd.dma_start(out=ksw[0:32, :], in_=kraw[32:64, :])
                    nc.gpsimd.dma_start(out=ksw[32:64, :], in_=kraw[0:32, :])
                    ka = npool.tile([ROPE, SC], F32, tag="ka", name=f"ka{sc}")
                    nc.vector.tensor_mul(ka[:], kraw[:ROPE, :], ck_t[:, ssl])
                    kb_ = npool.tile([ROPE, SC], F32, tag="kb", name=f"kb{sc}")
                    nc.vector.tensor_mul(kb_[:], ksw[:], sk_t[:, ssl])
                    ko = npool.tile([ROPE, SC], F32, tag="ko", name=f"ko{sc}")
                    nc.vector.tensor_add(ko[:], ka[:], kb_[:])
                    nc.gpsimd.dma_start(out=kpeT[:, ssl], in_=ko[:])

            if True:
                # ------------- Phase 2a-kv: K_nope / V projections (local data,
                # runs while the q AllGather is in flight)
                kv2pool = tc.tile_pool(name="kv2", bufs=1)
                kv2 = kv2pool.__enter__()
                KN = [kv2.tile([NOPE, S], FR, tag=f"kn{h}", name=f"kn{h}") for h in range(NHC)]
                V = [kv2.tile([P, NHC, VD], FR, tag=f"v{sb}", name=f"v{sb}") for sb in range(NKB)]
                kpe_sb = kv2.tile([ROPE, S], FR, tag="kpe")
                nc.gpsimd.dma_start(out=kpe_sb[:], in_=kpeT[:, :])
                with (
                    tc.tile_pool(name="whk", bufs=1) as whpool,
                    tc.tile_pool(name="acol2", bufs=1) as apool,
                    tc.tile_pool(name="ps2k", bufs=2, space="PSUM") as ps2pool,
                ):
                    wkvb_t = whpool.tile([P, NKVB, NHC * (NOPE + VD)], FR, tag="wkvb")
                    nc.gpsimd.dma_start(out=wkvb_t[:], in_=Wkvb3[:, :, :])
                    for sc in range(NSC):
                        ssl = slice(sc * SC, (sc + 1) * SC)
                        kvc = []
                        for j in range(NKVB):
                            t = apool.tile([P, SC], FR, tag=f"kv{j}", name=f"kvc{j}_{sc}")
                            nc.gpsimd.dma_start(out=t[:], in_=kvnT[j * P:(j + 1) * P, ssl])
                            kvc.append(t)
                        for h in range(NHC):
                            koff = h * (NOPE + VD)
                            ps = ps2pool.tile([P, SC], F32, tag="p2", name=f"p2k{h}_{sc}")
                            for j in range(NKVB):
                                nc.tensor.matmul(ps[:], lhsT=fr(wkvb_t[:, j, koff:koff + NOPE]),
                                                 rhs=kvc[j][:],
                                                 start=(j == 0), stop=(j == NKVB - 1))
                            nc.scalar.copy(KN[h][:, ssl], ps[:])
                            for sb in range(SC // P):
                                psv = ps2pool.tile([P, VD], F32, tag="pv", name=f"pv{h}_{sc}_{sb}")
                                for j in range(NKVB):
                                    nc.tensor.matmul(
                                        psv[:], lhsT=fr(kvc[j][:, sb * P:(sb + 1) * P]),
                                        rhs=fr(wkvb_t[:, j, koff + NOPE:koff + NOPE + VD]),
                                        start=(j == 0), stop=(j == NKVB - 1))
                                nc.scalar.copy(V[sc * (SC // P) + sb][:, h, :], psv[:])

                # ------------- Phase 2a-q: Q projections + rope (consumes the
                # AllGathered q_a_n, rank-chunked)
                with (
                    tc.tile_pool(name="whq", bufs=1) as whpool,
                    tc.tile_pool(name="acol", bufs=1) as apool,
                    tc.tile_pool(name="rope", bufs=2) as ropepool,
                    tc.tile_pool(name="ps2", bufs=2, space="PSUM") as ps2pool,
                ):
                    wqb_t = whpool.tile([P, NQB, NHC * QHD], FR, tag="wqb")
                    nc.gpsimd.dma_start(out=wqb_t[:], in_=Wqb3[:, :, :])
                    cq_t = whpool.tile([ROPE, S], F32, tag="cq")
                    sq_t = whpool.tile([ROPE, S], F32, tag="sq")
                    nc.gpsimd.dma_start(out=cq_t[:], in_=cq[:, :])
                    nc.gpsimd.dma_start(out=sq_t[:], in_=sq[:, :])
                    for r in range(NCORES):
                        csl = slice(r * SLC, (r + 1) * SLC)
                        qac = []
                        for j in range(NQB):
                            t = apool.tile([P, SLC], FR, tag=f"qa{j}", name=f"qac{j}_{r}")
                            nc.gpsimd.dma_start(out=t[:], in_=cc_q_out[r, j * P:(j + 1) * P, :])
                            qac.append(t)
                        for h in range(NHC):
                            qoff = h * QHD
                            ps = ps2pool.tile([P, SLC], F32, tag="p2", name=f"p2q{h}_{r}")
                            for j in range(NQB):
                                nc.tensor.matmul(ps[:], lhsT=fr(wqb_t[:, j, qoff:qoff + NOPE]),
                                                 rhs=qac[j][:],
                                                 start=(j == 0), stop=(j == NQB - 1))
                            qns = ropepool.tile([NOPE, SLC], F32, tag="qns", name=f"qns{h}_{r}")
                            nc.scalar.copy(qns[:], ps[:])
                            nc.gpsimd.dma_start(out=qnT[h * NOPE:(h + 1) * NOPE, csl], in_=qns[:])
                            ps64 = ps2pool.tile([ROPE, SLC], F32, tag="p64", name=f"p64q{h}_{r}")
                            for j in range(NQB):
                                nc.tensor.matmul(ps64[:], lhsT=fr(wqb_t[:, j, qoff + NOPE:qoff + QHD]),
                                                 rhs=qac[j][:],
                                                 start=(j == 0), stop=(j == NQB - 1))
                            qraw = ropepool.tile([ROPE, SLC], F32, tag="qraw", name=f"qraw{h}_{r}")
                            nc.scalar.copy(qraw[:], ps64[:])
                            qsw = ropepool.tile([ROPE, SLC], F32, tag="qsw", name=f"qsw{h}_{r}")
                            nc.gpsimd.dma_start(out=qsw[0:32, :], in_=qraw[32:64, :])
                            nc.gpsimd.dma_start(out=qsw[32:64, :], in_=qraw[0:32, :])
                            qa_ = ropepool.tile([ROPE, SLC], F32, tag="qa_", name=f"qa_{h}_{r}")
                            nc.vector.tensor_mul(qa_[:], qraw[:], cq_t[:, csl])
                            qb_ = ropepool.tile([ROPE, SLC], F32, tag="qb_", name=f"qb_{h}_{r}")
                            nc.vector.tensor_mul(qb_[:], qsw[:], sq_t[:, csl])
                            qrs = ropepool.tile([ROPE, SLC], F32, tag="qrs", name=f"qrs{h}_{r}")
                            nc.vector.tensor_add(qrs[:], qa_[:], qb_[:])
                            nc.gpsimd.dma_start(out=qrT[h * ROPE:(h + 1) * ROPE, csl], in_=qrs[:])

                # ------------- Phase 2b: attention
                with (
                    tc.tile_pool(name="att", bufs=2) as attpool,
                    tc.tile_pool(name="den", bufs=1) as denpool,
                    tc.tile_pool(name="ps_o", bufs=1, space="PSUM") as psopool,
                    tc.tile_pool(name="ps_l", bufs=2, space="PSUM") as pslpool,
                    tc.tile_pool(name="ps_d", bufs=1, space="PSUM") as psdpool,
                ):
                    for qc in range(NSC):
                        qsl = slice(qc * SC, (qc + 1) * SC)
                        kb_hi = (qc * 4 + 4) if causal else NKB
                        ops = [psopool.tile([VD, SC], F32, tag=f"o{h}", name=f"o{h}_{qc}") for h in range(NHC)]
                        dens = [denpool.tile([P, SC], FR, tag=f"d{h}", name=f"d{h}_{qc}") for h in range(NHC)]
                        qn_s, qr_s = [], []
                        for h in range(NHC):
                            qt = denpool.tile([NOPE, SC], FR, tag=f"qns{h}", name=f"qnl{h}_{qc}")
                            nc.gpsimd.dma_start(out=qt[:], in_=qnT[h * NOPE:(h + 1) * NOPE, qsl])
                            qn_s.append(qt)
                            qt2 = denpool.tile([ROPE, SC], FR, tag=f"qrs{h}", name=f"qrl{h}_{qc}")
                            nc.gpsimd.dma_start(out=qt2[:], in_=qrT[h * ROPE:(h + 1) * ROPE, qsl])
                            qr_s.append(qt2)
                        for kb in range(kb_hi):
                            ksl = slice(kb * P, (kb + 1) * P)
                            mt = attpool.tile([P, SC], F32, tag="mt", name=f"mt{qc}_{kb}")
                            nc.gpsimd.dma_start(out=mt[:], in_=maskT[ksl, qsl])
                            for h in range(NHC):
                                pl = pslpool.tile([P, SC], F32, tag="pl", name=f"pl{qc}_{kb}_{h}")
                                nc.tensor.matmul(pl[:], lhsT=KN[h][:, ksl], rhs=qn_s[h][:],
                                                 start=True, stop=False)
                                nc.tensor.matmul(pl[:], lhsT=kpe_sb[:, ksl], rhs=qr_s[h][:],
                                                 start=False, stop=True)
                                pe_ = attpool.tile([P, SC], F32, tag="pe", name=f"pe{qc}_{kb}_{h}")
                                nc.vector.tensor_add(pe_[:], pl[:], mt[:])
                                px = attpool.tile([P, SC], FR, tag="px", name=f"px{qc}_{kb}_{h}")
                                nc.scalar.activation(px[:], pe_[:], AF.Exp)
                                if kb == 0:
                                    nc.vector.tensor_copy(dens[h][:], px[:])
                                else:
                                    nc.vector.tensor_add(dens[h][:], dens[h][:], px[:])
                                nc.tensor.matmul(ops[h][:], lhsT=fr(V[kb][:, h, :]), rhs=px[:],
                                                 start=(kb == 0), stop=(kb == kb_hi - 1))
                        for h in range(NHC):
                            dps = psdpool.tile([1, SC], F32, tag="dps", name=f"dps{qc}_{h}")
                            nc.tensor.matmul(dps[:], lhsT=ones_t[:], rhs=dens[h][:],
                                             start=True, stop=True)
                            dsb = attpool.tile([1, SC], F32, tag="dsb", name=f"dsb{qc}_{h}")
                            nc.scalar.copy(dsb[:], dps[:])
                            rcp = attpool.tile([1, SC], FR, tag="rcp", name=f"rcp{qc}_{h}")
                            with nc.allow_low_precision(reason="f32r rounding for broadcast matmul"):
                                nc.vector.reciprocal(rcp[:], dsb[:])
                            bps2 = psdpool.tile([VD, SC], F32, tag="bps2", name=f"bps2{qc}_{h}")
                            nc.tensor.matmul(bps2[:], lhsT=ones_row[:], rhs=rcp[:],
                                             start=True, stop=True)
                            rbb = attpool.tile([VD, SC], F32, tag="rbb", name=f"rbb{qc}_{h}")
                            nc.scalar.copy(rbb[:], bps2[:])
                            on_ = attpool.tile([VD, SC], F32, tag="on", name=f"on{qc}_{h}")
                            nc.vector.tensor_mul(on_[:], ops[h][:], rbb[:])
                            nc.gpsimd.dma_start(out=onT[h * VD:(h + 1) * VD, qsl], in_=on_[:])
                kv2pool.__exit__(None, None, None)

            # ------------- Phase 3: output projection (partial over head slice)
            with (
                tc.tile_pool(name="wo", bufs=1) as wopool,
                tc.tile_pool(name="oc", bufs=1) as ocpool,
                tc.tile_pool(name="oo", bufs=3) as oopool,
                tc.tile_pool(name="po", bufs=3, space="PSUM") as popool,
            ):
                wo_t = wopool.tile([P, NKVB, H], FR, tag="wo")
                nc.gpsimd.dma_start(out=wo_t[:], in_=Wo3[:, :, :])
                for sc in range(NSC):
                    ssl = slice(sc * SC, (sc + 1) * SC)
                    ocs = []
                    for j in range(NKVB):
                        t = ocpool.tile([P, SC], FR, tag=f"oc{j}", name=f"oc{j}_{sc}")
                        nc.gpsimd.dma_start(out=t[:], in_=onT[j * P:(j + 1) * P, ssl])
                        ocs.append(t)
                    for ho in range(H // P):
                        ps = popool.tile([P, SC], F32, tag="po", name=f"po{sc}_{ho}")
                        for j in range(NKVB):
                            nc.tensor.matmul(ps[:], lhsT=fr(wo_t[:, j, ho * P:(ho + 1) * P]),
                                             rhs=ocs[j][:], start=(j == 0), stop=(j == NKVB - 1))
                        ot = oopool.tile([P, SC], F32, tag="ot", name=f"ot{sc}_{ho}")
                        nc.scalar.copy(ot[:], ps[:])
                        nc.gpsimd.dma_start(out=outT[ho * P:(ho + 1) * P, ssl], in_=ot[:])

    split_multiwaits(nc)
    return nc


def _pack_front(WqaT, WkvaT):
    """[4096, 1536+576] -> [128, 17*32, 128], zero-padded rope block."""
    Wfull = np.concatenate([WqaT, WkvaT], axis=1)
    out = np.zeros((P, N_FB * N_KI, P), np.float32)
    off = 0
    for fb, w in enumerate(FB_W):
        blk = Wfull[:, off:off + w].reshape(N_KI, P, w).transpose(1, 0, 2)
        out[:, fb * N_KI:(fb + 1) * N_KI, :w] = blk
        off += w
    return np.ascontiguousarray(out.reshape(P, -1))


def _pack_k(WT, nhw):
    """[K, nhw] -> [128, (K//128)*nhw]: k-tile-major packing of a T-layout weight."""
    K = WT.shape[0]
    t = WT.reshape(K // P, P, nhw).transpose(1, 0, 2).reshape(P, (K // P) * nhw)
    return np.ascontiguousarray(t, np.float32)


def _rope_tables():
    inv = 1.0 / (BASE ** (np.arange(0, ROPE, 2, dtype=np.float64) / ROPE))
    t = np.arange(S, dtype=np.float64)
    fr_ = np.outer(t, inv)
    emb = np.concatenate([fr_, fr_], axis=1)
    cos = np.cos(emb).T.astype(np.float32)
    sin = np.sin(emb).T.astype(np.float32)
    ssin = sin.copy()
    ssin[:32] *= -1.0
    return cos, ssin


def kernel(hidden_states, attention_mask, Wqa, qa_ln_w, Wqb, Wkva, kva_ln_w, Wkvb, Wo):
    hidden_states = np.asarray(hidden_states, np.float32)
    attention_mask = np.asarray(attention_mask, np.float32)
    Wqa = np.asarray(Wqa, np.float32)
    Wqb = np.asarray(Wqb, np.float32)
    Wkva = np.asarray(Wkva, np.float32)
    Wkvb = np.asarray(Wkvb, np.float32)
    Wo = np.asarray(Wo, np.float32)
    qa_ln_w = np.asarray(qa_ln_w, np.float32)
    kva_ln_w = np.asarray(kva_ln_w, np.float32)

    mask = attention_mask[0, 0]
    tril = np.tril(np.ones((S, S), bool))
    causal = bool(np.array_equal(mask, np.where(tril, 0.0, -1e9).astype(np.float32)))

    hT = np.ascontiguousarray(hidden_states[0].T)
    maskT = np.ascontiguousarray(mask.T)
    Wp = _pack_front(np.ascontiguousarray(Wqa.T), np.ascontiguousarray(Wkva.T))
    cos, ssin = _rope_tables()

    Wqb_eff = (Wqb * qa_ln_w[None, :]).astype(np.float32) * np.float32(SCALE)
    Wkvb_eff = (Wkvb * kva_ln_w[None, :]).astype(np.float32)

    in_maps = []
    for c in range(NCORES):
        hsl = slice(c * NHC * QHD, (c + 1) * NHC * QHD)
        ksl = slice(c * NHC * (NOPE + VD), (c + 1) * NHC * (NOPE + VD))
        osl = slice(c * NHC * VD, (c + 1) * NHC * VD)
        in_maps.append({
            "hT": hT, "maskT": maskT, "Wp": Wp,
            "hTs": np.ascontiguousarray(hT[:, c * SLC:(c + 1) * SLC]),
            "Wqb_p": _pack_k(np.ascontiguousarray(Wqb_eff[hsl].T), NHC * QHD),
            "Wkvb_p": _pack_k(np.ascontiguousarray(Wkvb_eff[ksl].T), NHC * (NOPE + VD)),
            "Wo_p": _pack_k(np.ascontiguousarray(Wo[:, osl].T), H),
            "cq": cos, "sq": ssin,
        })

    nc = build(causal)
    trace = bool(os.environ.get("KPROF"))
    res = run_bass_kernel_spmd(nc, in_maps, list(range(NCORES)), trace=trace)
    if trace:
        global LAST_RES
        LAST_RES = res
        print(f"HW exec time: {res.exec_time_ns} ns (mean {res.mean_exec_time_ns}, "
              f"max core {res.max_exec_time_core_id})")
    acc = res.results[0]["outT"].copy()
    for c in range(1, NCORES):
        acc += res.results[c]["outT"]
    return np.ascontiguousarray(acc.T)[None, :, :].astype(np.float32)



# revision 21
# speedup vs baseline: 2.0957x; 2.0957x over previous
"""DeepseekV2 MLA attention (B=1, S=2048, H=4096, NH=32) on 8 TRN2 cores.

Sharding: tensor-parallel over heads (4 heads/core).  Both front projections
(q_a and kv_a) run data-parallel over sequence (each core does its 256-token
slice) and are AllGathered in bf16.  Each core emits a partial output
projection (its head slice of Wo); the host sums the 8 bf16 partials in f32.

All matmuls run with bf16 operands (f32 PSUM accumulation) — end-to-end rel
err ~5e-3 vs the 2e-2 gate.  Weights are host-packed into k-tile-major
layouts so every weight DMA has multi-KB contiguous rows.  DMAs are issued on
the sync/scalar HWDGE queues (hardware descriptor generation) instead of
gpsimd SWDGE.  Attention runs logits^T [k, q] with softmax over the partition
axis; causal masking skips above-diagonal key blocks entirely and applies 4
constant diagonal-pattern tiles (no mask traffic); below-diagonal blocks take
exp() straight out of PSUM.  Denominators accumulate on the vector engine and
the (slow) vector reciprocal is batched 4 heads at a time.
"""

import ctypes
import os
import numpy as np
import ml_dtypes

import concourse.bass as bass
import concourse.mybir as mybir
from concourse.tile import TileContext
import concourse.bass_utils as bass_utils
from concourse.bass_utils import run_bass_kernel_spmd

bass_utils.upload_artifacts = lambda tmpdir: tmpdir  # no artifact bucket here

S = 2048
H = 4096
NCORES = 8
NHC = 4            # heads per core
NOPE, ROPE, VD = 128, 64, 128
QHD = NOPE + ROPE  # 192
QLR, KVLR = 1536, 512
BASE = 10000.0
EPS = 1e-6
SCALE = QHD ** -0.5
P = 128
SC = 512           # seq chunk
SLC = S // NCORES  # 256, per-core front slice
NSC = S // SC      # 4
NKB = S // P       # 16 key blocks
N_KI = H // P      # 32 front contraction tiles
NQB = QLR // P     # 12
NKVB = KVLR // P   # 4
NFB = 5 + NQB      # 17 front blocks: 4 c_kv + 1 k_pe(64) + 12 q
F32 = mybir.dt.float32
FR = mybir.dt.float32r
BF = mybir.dt.bfloat16
AF = mybir.ActivationFunctionType
NPBF = ml_dtypes.bfloat16

LAST_RES = None


def axon_reset():
    import jax
    jax.devices()
    lib = ctypes.CDLL('/opt/axon/libaxon_pjrt.so')
    lib.axon_reset.restype = ctypes.c_int64
    return lib.axon_reset()


def split_multiwaits(nc, cap=1):
    """Allow only `cap` sync-waits per instruction; spill extras onto
    same-engine NoOps inserted just before the instruction."""
    for f in nc.m.functions:
        for b in f.blocks:
            li = b.instructions
            out = []
            changed = False
            for inst in list(li):
                si = getattr(inst, "sync_info", None)
                waits = list(si.on_wait) if si is not None and si.on_wait else []
                if len(waits) > cap:
                    changed = True
                    extra, keep = waits[:-cap], waits[-cap:]
                    for j in range(0, len(extra), cap):
                        out.append(mybir.InstNoOp(
                            name=nc.get_next_instruction_name(),
                            engine=inst.engine, ins=[], outs=[],
                            sync_info=mybir.SyncInfo(
                                on_wait=extra[j:j + cap], on_update=[]),
                            bass_nofuse=True,
                        ))
                    inst.sync_info = mybir.SyncInfo(
                        on_wait=keep, on_update=list(si.on_update))
                out.append(inst)
            if changed:
                li[:] = out


def build(causal: bool) -> bass.Bass:
    nc = bass.Bass()
    hp = nc.declare_dram_parameter("hp", [P, N_KI * SLC], BF, isOutput=False)
    wf = nc.declare_dram_parameter("wf", [P, NFB * N_KI * P], BF, isOutput=False)
    wqb = nc.declare_dram_parameter("wqb", [P, NQB * 768], BF, isOutput=False)
    wkvb = nc.declare_dram_parameter("wkvb", [P, NKVB * 1024], BF, isOutput=False)
    wo = nc.declare_dram_parameter("wo", [P, NKVB * H], BF, isOutput=False)
    cq2 = nc.declare_dram_parameter("cq2", [P, S], BF, isOutput=False)
    sq2 = nc.declare_dram_parameter("sq2", [P, S], BF, isOutput=False)
    cqs = nc.declare_dram_parameter("cqs", [ROPE, SLC], BF, isOutput=False)
    sqs = nc.declare_dram_parameter("sqs", [ROPE, SLC], BF, isOutput=False)
    mdg = nc.declare_dram_parameter("mdg", [P, 4 * SC], BF, isOutput=False)
    maskT = nc.declare_dram_parameter("maskT", [S, S], BF, isOutput=False)
    outT = nc.declare_dram_parameter("outT", [H, S], BF, isOutput=True)

    hp3 = hp.rearrange("p (k s) -> p k s", k=N_KI)
    wf4 = wf.rearrange("p (g k w) -> p g k w", g=NFB, k=N_KI)
    wqb3 = wqb.rearrange("p (j w) -> p j w", j=NQB)
    wkvb3 = wkvb.rearrange("p (j w) -> p j w", j=NKVB)
    wo3 = wo.rearrange("p (j w) -> p j w", j=NKVB)
    mdg3 = mdg.rearrange("p (m s) -> p m s", m=4)

    def fr(ap):
        return ap.bitcast(FR)

    with TileContext(nc) as tc:
        with (
            tc.tile_pool(name="dram", bufs=1, space="DRAM") as dpool,
            tc.tile_pool(name="const", bufs=1) as cpool,
        ):
            cc_q_in = dpool.tile([QLR, SLC], BF)
            cc_q_out = dpool.tile([NCORES, QLR, SLC], BF, addr_space="Shared")
            cc_kv_in = dpool.tile([KVLR + ROPE, SLC], BF)
            cc_kv_out = dpool.tile([NCORES, KVLR + ROPE, SLC], BF, addr_space="Shared")

            ones_f = cpool.tile([P, 1], F32)
            nc.vector.memset(ones_f[:], 1.0)
            ones_rf = cpool.tile([1, P], F32)
            nc.vector.memset(ones_rf[:], 1.0)
            ones_t = cpool.tile([P, 1], FR)
            nc.scalar.copy(ones_t[:], ones_f[:])
            ones_row = cpool.tile([1, P], FR)
            nc.scalar.copy(ones_row[:], ones_rf[:])

            # big prefetches on the Activation HWDGE queue
            wqb_t = cpool.tile([P, NQB, 768], BF)
            nc.scalar.dma_start(out=wqb_t[:], in_=wqb3[:, :, :])
            wkvb_t = cpool.tile([P, NKVB, 1024], BF)
            nc.scalar.dma_start(out=wkvb_t[:], in_=wkvb3[:, :, :])
            cq_t = cpool.tile([P, S], BF)
            nc.scalar.dma_start(out=cq_t[:], in_=cq2[:, :])
            sq_t = cpool.tile([P, S], BF)
            nc.scalar.dma_start(out=sq_t[:], in_=sq2[:, :])
            mdg_t = cpool.tile([P, 4, SC], BF)
            nc.scalar.dma_start(out=mdg_t[:], in_=mdg3[:, :, :])

            # persistent activation tiles
            KN = [cpool.tile([NOPE, S], BF, name=f"KN{h}") for h in range(NHC)]
            V = [cpool.tile([P, NHC * VD], BF, name=f"V{i}") for i in range(NKB)]
            kpe2 = cpool.tile([P, S], BF)
            qn = [cpool.tile([NOPE, S], BF, name=f"qn{h}") for h in range(NHC)]
            qr = [cpool.tile([P, S], BF, name=f"qr{i}") for i in range(2)]

            # ------------- Phase 1: fronts (kv first, then q) + AllGathers
            with (
                tc.tile_pool(name="hpool", bufs=1) as hpool,
                tc.tile_pool(name="wfp", bufs=3) as wpool,
                tc.tile_pool(name="raw", bufs=1) as rpool,
                tc.tile_pool(name="nrm", bufs=2) as npool,
                tc.tile_pool(name="psf", bufs=3, space="PSUM") as pspool,
                tc.tile_pool(name="ps1", bufs=1, space="PSUM") as ps1pool,
            ):
                hp_t = hpool.tile([P, N_KI, SLC], BF)
                nc.sync.dma_start(out=hp_t[:], in_=hp3[:, :, :])

                def front_block(g, w, name):
                    wt = wpool.tile([P, N_KI, P], BF, tag="w", name=f"wf{name}")
                    nc.sync.dma_start(out=wt[:], in_=wf4[:, g, :, :])
                    ps = pspool.tile([P, SLC], F32, tag="ps", name=f"psf{name}")
                    for ki in range(N_KI):
                        nc.tensor.matmul(ps[:w, :], lhsT=wt[:, ki, :w],
                                         rhs=hp_t[:, ki, :],
                                         start=(ki == 0), stop=(ki == N_KI - 1))
                    raw = rpool.tile([P, SLC], BF, tag=f"r{name}", name=f"raw{name}")
                    with nc.allow_low_precision(reason="bf16 activations"):
                        nc.scalar.copy(raw[:w, :], ps[:w, :])
                    return raw

                def rms_apply(sq_ps, raws, n_feat, nblocks, cc_dst, name):
                    ms = npool.tile([1, SLC], F32, tag="ms", name=f"ms{name}")
                    nc.scalar.activation(ms[:], sq_ps[:], AF.Copy,
                                         scale=1.0 / n_feat, bias=EPS)
                    rc = npool.tile([1, SLC], F32, tag="rc", name=f"rc{name}")
                    nc.vector.reciprocal(rc[:], ms[:])
                    rs = npool.tile([1, SLC], FR, tag="rs", name=f"rs{name}")
                    nc.scalar.activation(rs[:], rc[:], AF.Sqrt)
                    bps = ps1pool.tile([P, SLC], F32, tag="bps", name=f"bps{name}")
                    nc.tensor.matmul(bps[:], lhsT=ones_row[:], rhs=rs[:],
                                     start=True, stop=True)
                    rb = npool.tile([P, SLC], F32, tag="rb", name=f"rb{name}")
                    nc.scalar.copy(rb[:], bps[:])
                    for j in range(nblocks):
                        nt = npool.tile([P, SLC], BF, tag="nt", name=f"nt{name}{j}")
                        with nc.allow_low_precision(reason="bf16 activations"):
                            nc.vector.tensor_mul(nt[:], raws[j][:], rb[:])
                        nc.sync.dma_start(out=cc_dst[j * P:(j + 1) * P, :], in_=nt[:])

                # kv front: blocks 0..3 = c_kv, 4 = k_pe
                kvraws = []
                sq_kv = ps1pool.tile([1, SLC], F32, tag="sqkv")
                for g in range(4):
                    raw = front_block(g, P, f"kv{g}")
                    kvraws.append(raw)
                    sqt = npool.tile([P, SLC], FR, tag="sqt", name=f"sqtk{g}")
                    nc.vector.tensor_mul(sqt[:], raw[:], raw[:])
                    nc.tensor.matmul(sq_kv[:], lhsT=ones_t[:], rhs=sqt[:],
                                     start=(g == 0), stop=(g == 3))
                kraw = front_block(4, ROPE, "kpe")
                rms_apply(sq_kv, kvraws, KVLR, NKVB, cc_kv_in, "kv")
                # rope on k_pe (local slice, per-core tables)
                ck_t = npool.tile([ROPE, SLC], BF, tag="ck")
                nc.sync.dma_start(out=ck_t[:], in_=cqs[:, :])
                sk_t = npool.tile([ROPE, SLC], BF, tag="sk")
                nc.sync.dma_start(out=sk_t[:], in_=sqs[:, :])
                ksw = npool.tile([ROPE, SLC], BF, tag="ksw")
                nc.sync.dma_start(out=ksw[0:32, :], in_=kraw[32:64, :])
                nc.sync.dma_start(out=ksw[32:64, :], in_=kraw[0:32, :])
                ka = npool.tile([ROPE, SLC], F32, tag="ka")
                nc.vector.tensor_mul(ka[:], kraw[:ROPE, :], ck_t[:])
                kb_ = npool.tile([ROPE, SLC], F32, tag="kb")
                nc.vector.tensor_mul(kb_[:], ksw[:], sk_t[:])
                ko = npool.tile([ROPE, SLC], BF, tag="ko")
                with nc.allow_low_precision(reason="bf16 activations"):
                    nc.vector.tensor_add(ko[:], ka[:], kb_[:])
                nc.sync.dma_start(out=cc_kv_in[KVLR:KVLR + ROPE, :], in_=ko[:])
                nc.gpsimd.collective_compute(
                    "AllGather", mybir.AluOpType.bypass,
                    replica_groups=[list(range(NCORES))],
                    ins=[cc_kv_in.opt()], outs=[cc_kv_out.opt()])

                # q front: blocks 5..16
                qraws = []
                sq_q = ps1pool.tile([1, SLC], F32, tag="sqq")
                for j in range(NQB):
                    raw = front_block(5 + j, P, f"q{j}")
                    qraws.append(raw)
                    sqt = npool.tile([P, SLC], FR, tag="sqt", name=f"sqtq{j}")
                    nc.vector.tensor_mul(sqt[:], raw[:], raw[:])
                    nc.tensor.matmul(sq_q[:], lhsT=ones_t[:], rhs=sqt[:],
                                     start=(j == 0), stop=(j == NQB - 1))
                rms_apply(sq_q, qraws, QLR, NQB, cc_q_in, "q")
                nc.gpsimd.collective_compute(
                    "AllGather", mybir.AluOpType.bypass,
                    replica_groups=[list(range(NCORES))],
                    ins=[cc_q_in.opt()], outs=[cc_q_out.opt()])

            # ------------- Phase 2: K/V up-projection (consumes kv AllGather)
            with (
                tc.tile_pool(name="kvin", bufs=2) as kvip,
                tc.tile_pool(name="psK", bufs=2, space="PSUM") as pskp,
                tc.tile_pool(name="psV", bufs=2, space="PSUM") as psvp,
            ):
                for sc in range(NSC):
                    ssl = slice(sc * SC, (sc + 1) * SC)
                    kvc = []
                    for j in range(NKVB):
                        t = kvip.tile([P, SC], BF, tag=f"kv{j}", name=f"kvc{j}_{sc}")
                        for rr in range(2):
                            r = 2 * sc + rr
                            nc.sync.dma_start(
                                out=t[:, rr * SLC:(rr + 1) * SLC],
                                in_=cc_kv_out[r, j * P:(j + 1) * P, :])
                        kvc.append(t)
                    for rr in range(2):
                        r = 2 * sc + rr
                        csl = slice(sc * SC + rr * SLC, sc * SC + (rr + 1) * SLC)
                        nc.sync.dma_start(out=kpe2[0:ROPE, csl],
                                          in_=cc_kv_out[r, KVLR:KVLR + ROPE, :])
                        nc.sync.dma_start(out=kpe2[ROPE:P, csl],
                                          in_=cc_kv_out[r, KVLR:KVLR + ROPE, :])
                    for h in range(NHC):
                        ps = pskp.tile([P, SC], F32, tag="pk", name=f"psk{h}_{sc}")
                        for j in range(NKVB):
                            nc.tensor.matmul(ps[:], lhsT=wkvb_t[:, j, h * P:(h + 1) * P],
                                             rhs=kvc[j][:],
                                             start=(j == 0), stop=(j == NKVB - 1))
                        with nc.allow_low_precision(reason="bf16 activations"):
                            nc.scalar.copy(KN[h][:, ssl], ps[:])
                    for kb in range(SC // P):
                        psv = psvp.tile([P, SC], F32, tag="pv", name=f"psv{kb}_{sc}")
                        for j in range(NKVB):
                            nc.tensor.matmul(psv[:], lhsT=kvc[j][:, kb * P:(kb + 1) * P],
                                             rhs=wkvb_t[:, j, 512:1024],
                                             start=(j == 0), stop=(j == NKVB - 1))
                        with nc.allow_low_precision(reason="bf16 activations"):
                            nc.scalar.copy(V[sc * 4 + kb][:], psv[:])

            # ------------- Phase 3: Q up-projection + rope (consumes q AllGather)
            with (
                tc.tile_pool(name="qin", bufs=2) as qip,
                tc.tile_pool(name="rope", bufs=2) as ropool,
                tc.tile_pool(name="psQ", bufs=2, space="PSUM") as psqp,
                tc.tile_pool(name="psR", bufs=2, space="PSUM") as psrp,
            ):
                for sc in range(NSC):
                    ssl = slice(sc * SC, (sc + 1) * SC)
                    qac = []
                    for j in range(NQB):
                        t = qip.tile([P, SC], BF, tag=f"qa{j}", name=f"qac{j}_{sc}")
                        for rr in range(2):
                            r = 2 * sc + rr
                            nc.sync.dma_start(
                                out=t[:, rr * SLC:(rr + 1) * SLC],
                                in_=cc_q_out[r, j * P:(j + 1) * P, :])
                        qac.append(t)
                    for h in range(NHC):
                        ps = psqp.tile([P, SC], F32, tag="pq", name=f"psq{h}_{sc}")
                        for j in range(NQB):
                            nc.tensor.matmul(ps[:], lhsT=wqb_t[:, j, h * P:(h + 1) * P],
                                             rhs=qac[j][:],
                                             start=(j == 0), stop=(j == NQB - 1))
                        with nc.allow_low_precision(reason="bf16 activations"):
                            nc.scalar.copy(qn[h][:, ssl], ps[:])
                    for pr in range(2):
                        ps = psrp.tile([P, SC], F32, tag="pr", name=f"psr{pr}_{sc}")
                        for j in range(NQB):
                            nc.tensor.matmul(
                                ps[:], lhsT=wqb_t[:, j, 512 + pr * P:512 + (pr + 1) * P],
                                rhs=qac[j][:],
                                start=(j == 0), stop=(j == NQB - 1))
                        qraw = ropool.tile([P, SC], BF, tag="qraw", name=f"qraw{pr}_{sc}")
                        with nc.allow_low_precision(reason="bf16 activations"):
                            nc.scalar.copy(qraw[:], ps[:])
                        qsw = ropool.tile([P, SC], BF, tag="qsw", name=f"qsw{pr}_{sc}")
                        nc.sync.dma_start(out=qsw[0:32, :], in_=qraw[32:64, :])
                        nc.sync.dma_start(out=qsw[32:64, :], in_=qraw[0:32, :])
                        nc.sync.dma_start(out=qsw[64:96, :], in_=qraw[96:128, :])
                        nc.sync.dma_start(out=qsw[96:128, :], in_=qraw[64:96, :])
                        qa_ = ropool.tile([P, SC], BF, tag="qa_", name=f"qa_{pr}_{sc}")
                        qb_ = ropool.tile([P, SC], BF, tag="qb_", name=f"qb_{pr}_{sc}")
                        with nc.allow_low_precision(reason="bf16 activations"):
                            nc.vector.tensor_mul(qa_[:], qraw[:], cq_t[:, ssl])
                            nc.vector.tensor_mul(qb_[:], qsw[:], sq_t[:, ssl])
                            nc.vector.tensor_add(qr[pr][:, ssl], qa_[:], qb_[:])

            # ------------- Phases 4+5 shared tiles (space freed by phases 1-3)
            wopool = tc.alloc_tile_pool(name="wop", bufs=1)
            wo_t = wopool.tile([P, NKVB, H], BF)
            nc.scalar.dma_start(out=wo_t[:], in_=wo3[:, :, :])
            oc = [wopool.tile([VD, SC], BF, name=f"oc{i}") for i in range(NSC * NHC)]

            # ------------- Phase 4: attention
            with (
                tc.tile_pool(name="att", bufs=3) as attp,
                tc.tile_pool(name="pxp", bufs=3) as pxp,
                tc.tile_pool(name="den", bufs=2) as denp,
                tc.tile_pool(name="ocf", bufs=1) as ocfp,
                tc.tile_pool(name="psL", bufs=3, space="PSUM") as plp,
                tc.tile_pool(name="psO", bufs=2, space="PSUM") as opp,
                tc.tile_pool(name="psD", bufs=1, space="PSUM") as pdp,
            ):
                for qc in range(NSC):
                    qsl = slice(qc * SC, (qc + 1) * SC)
                    kb_hi = (4 * qc + 4) if causal else NKB
                    dsb4 = attp.tile([NHC, SC], F32, tag="dsb4", name=f"dsb4_{qc}")
                    ocf = [ocfp.tile([VD, SC], F32, tag=f"ocf{h}", name=f"ocf{h}_{qc}")
                           for h in range(NHC)]
                    for h in range(NHC):
                        pair, half = h // 2, h % 2
                        ops = opp.tile([VD, SC], F32, tag="o", name=f"ops{qc}_{h}")
                        dens = denp.tile([P, SC], FR, tag="d", name=f"den{qc}_{h}")
                        for kb in range(kb_hi):
                            ksl = slice(kb * P, (kb + 1) * P)
                            pl = plp.tile([P, SC], F32, tag="pl", name=f"pl{qc}_{h}_{kb}")
                            nc.tensor.matmul(pl[:], lhsT=KN[h][:, ksl],
                                             rhs=qn[h][:, qsl], start=True, stop=False)
                            nc.tensor.matmul(
                                pl[:], lhsT=kpe2[half * ROPE:(half + 1) * ROPE, ksl],
                                rhs=qr[pair][half * ROPE:(half + 1) * ROPE, qsl],
                                start=False, stop=True)
                            px = pxp.tile([P, SC], BF, tag="px", name=f"px{qc}_{h}_{kb}")
                            with nc.allow_low_precision(reason="bf16 softmax weights"):
                                if causal and kb >= 4 * qc:
                                    pe_ = attp.tile([P, SC], F32, tag="pe",
                                                    name=f"pe{qc}_{h}_{kb}")
                                    nc.vector.tensor_add(pe_[:], pl[:],
                                                         mdg_t[:, kb - 4 * qc, :])
                                    nc.scalar.activation(px[:], pe_[:], AF.Exp)
                                elif not causal:
                                    mt = attp.tile([P, SC], BF, tag="mt",
                                                   name=f"mt{qc}_{h}_{kb}")
                                    nc.sync.dma_start(out=mt[:], in_=maskT[ksl, qsl])
                                    pe_ = attp.tile([P, SC], F32, tag="pe",
                                                    name=f"pe{qc}_{h}_{kb}")
                                    nc.vector.tensor_add(pe_[:], pl[:], mt[:])
                                    nc.scalar.activation(px[:], pe_[:], AF.Exp)
                                else:
                                    nc.scalar.activation(px[:], pl[:], AF.Exp)
                            if kb == 0:
                                nc.vector.tensor_copy(dens[:], px[:])
                            else:
                                nc.vector.tensor_add(dens[:], dens[:], px[:])
                            nc.tensor.matmul(ops[:], lhsT=V[kb][:, h * VD:(h + 1) * VD],
                                             rhs=px[:],
                                             start=(kb == 0), stop=(kb == kb_hi - 1))
                        # head epilogue: den row-sum + stash; free the PSUM tile
                        dps = pdp.tile([1, SC], F32, tag="dp", name=f"dps{qc}_{h}")
                        nc.tensor.matmul(dps[:], lhsT=ones_t[:], rhs=dens[:],
                                         start=True, stop=True)
                        dtmp = attp.tile([1, SC], F32, tag="dtmp", name=f"dtmp{qc}_{h}")
                        nc.scalar.copy(dtmp[:], dps[:])
                        nc.sync.dma_start(out=dsb4[h:h + 1, :], in_=dtmp[:])
                        nc.scalar.copy(ocf[h][:], ops[:])
                    # batched reciprocal over 4 heads, then scale
                    rc4 = attp.tile([NHC, SC], FR, tag="rc4", name=f"rc4_{qc}")
                    with nc.allow_low_precision(reason="f32r for broadcast matmul"):
                        nc.vector.reciprocal(rc4[:], dsb4[:])
                    for h in range(NHC):
                        rr_ = attp.tile([1, SC], FR, tag="rr", name=f"rr{qc}_{h}")
                        nc.sync.dma_start(out=rr_[:], in_=rc4[h:h + 1, :])
                        bps2 = pdp.tile([VD, SC], F32, tag="bc", name=f"bps2{qc}_{h}")
                        nc.tensor.matmul(bps2[:], lhsT=ones_row[:], rhs=rr_[:],
                                         start=True, stop=True)
                        rbb = attp.tile([VD, SC], F32, tag="rbb", name=f"rbb{qc}_{h}")
                        nc.scalar.copy(rbb[:], bps2[:])
                        with nc.allow_low_precision(reason="bf16 activations"):
                            nc.vector.tensor_mul(oc[qc * NHC + h][:], ocf[h][:], rbb[:])

            # ------------- Phase 5: output projection (partial over head slice)
            with (
                tc.tile_pool(name="oo", bufs=3) as oop,
                tc.tile_pool(name="psW", bufs=3, space="PSUM") as pop,
            ):
                for sc in range(NSC):
                    ssl = slice(sc * SC, (sc + 1) * SC)
                    for ho in range(H // P):
                        ps = pop.tile([P, SC], F32, tag="po", name=f"po{sc}_{ho}")
                        for j in range(NKVB):
                            nc.tensor.matmul(ps[:], lhsT=wo_t[:, j, ho * P:(ho + 1) * P],
                                             rhs=oc[sc * NHC + j][:],
                                             start=(j == 0), stop=(j == NKVB - 1))
                        ot = oop.tile([P, SC], BF, tag="ot", name=f"ot{sc}_{ho}")
                        with nc.allow_low_precision(reason="bf16 partial output"):
                            nc.scalar.copy(ot[:], ps[:])
                        nc.sync.dma_start(out=outT[ho * P:(ho + 1) * P, ssl], in_=ot[:])
            wopool.release()

    split_multiwaits(nc)
    return nc


def _pack_front(Wqa, Wkva):
    """-> [128, 17*32*128] bf16, blocks: 4 c_kv, 1 k_pe(64,padded), 12 q."""
    out = np.zeros((P, NFB, N_KI, P), np.float32)
    blocks = [Wkva[g * P:(g + 1) * P] for g in range(4)]
    blocks.append(Wkva[KVLR:KVLR + ROPE])
    blocks += [Wqa[j * P:(j + 1) * P] for j in range(NQB)]
    for g, B in enumerate(blocks):
        w = B.shape[0]
        t = B.T.reshape(N_KI, P, w).transpose(1, 0, 2)
        out[:, g, :, :w] = t
    return np.ascontiguousarray(out.reshape(P, -1)).astype(NPBF)


def _pack_ktile(WT, nout):
    """[K, nout] (K contraction) -> [128, (K//128)*nout] bf16 k-tile-major."""
    K = WT.shape[0]
    t = WT.reshape(K // P, P, nout).transpose(1, 0, 2).reshape(P, (K // P) * nout)
    return np.ascontiguousarray(t).astype(NPBF)


def _rope_tables():
    inv = 1.0 / (BASE ** (np.arange(0, ROPE, 2, dtype=np.float64) / ROPE))
    t = np.arange(S, dtype=np.float64)
    fr_ = np.outer(t, inv)
    emb = np.concatenate([fr_, fr_], axis=1)
    cos = np.cos(emb).T.astype(np.float32)   # [64, S]
    ssin = np.sin(emb).T.astype(np.float32)
    ssin[:32] *= -1.0
    return cos, ssin


def kernel(hidden_states, attention_mask, Wqa, qa_ln_w, Wqb, Wkva, kva_ln_w, Wkvb, Wo):
    hidden_states = np.asarray(hidden_states, np.float32)
    attention_mask = np.asarray(attention_mask, np.float32)
    Wqa = np.asarray(Wqa, np.float32)
    Wqb = np.asarray(Wqb, np.float32)
    Wkva = np.asarray(Wkva, np.float32)
    Wkvb = np.asarray(Wkvb, np.float32)
    Wo = np.asarray(Wo, np.float32)
    qa_ln_w = np.asarray(qa_ln_w, np.float32)
    kva_ln_w = np.asarray(kva_ln_w, np.float32)

    mask = attention_mask[0, 0]
    tril = np.tril(np.ones((S, S), bool))
    causal = bool(np.array_equal(mask, np.where(tril, 0.0, -1e9).astype(np.float32)))

    hT = np.ascontiguousarray(hidden_states[0].T)           # [H, S]
    maskT = np.ascontiguousarray(mask.T).astype(NPBF)
    wf = _pack_front(Wqa, Wkva)
    cos, ssin = _rope_tables()
    cq2 = np.concatenate([cos, cos], axis=0).astype(NPBF)   # [128, S]
    sq2 = np.concatenate([ssin, ssin], axis=0).astype(NPBF)

    # diagonal causal mask tiles: mdg[i, m, j] = 0 if 128*m+i <= j else -1e9
    ii = np.arange(P)[:, None, None]
    mm_ = np.arange(4)[None, :, None]
    jj = np.arange(SC)[None, None, :]
    mdg = np.where(P * mm_ + ii <= jj, 0.0, -1e9).astype(np.float32).reshape(P, -1).astype(NPBF)

    Wqb_eff = (Wqb * qa_ln_w[None, :]).astype(np.float32) * np.float32(SCALE)
    Wkvb_eff = (Wkvb * kva_ln_w[None, :]).astype(np.float32)

    in_maps = []
    for c in range(NCORES):
        csl = slice(c * SLC, (c + 1) * SLC)
        # hp: [H, SLC] -> [128, 32*SLC]
        hs = hT[:, csl].reshape(N_KI, P, SLC).transpose(1, 0, 2)
        hp = np.ascontiguousarray(hs.reshape(P, -1)).astype(NPBF)
        # wqb: rows = 4 heads x (nope128+rope64); cols reordered
        Wc = Wqb_eff[c * NHC * QHD:(c + 1) * NHC * QHD]     # [768, QLR]
        nw = np.zeros((768, QLR), np.float32)
        for h in range(NHC):
            nw[h * P:(h + 1) * P] = Wc[h * QHD:h * QHD + NOPE]
        for pr in range(2):
            for hh in range(2):
                h = pr * 2 + hh
                nw[512 + pr * P + hh * ROPE: 512 + pr * P + (hh + 1) * ROPE] = \
                    Wc[h * QHD + NOPE:(h + 1) * QHD]
        # wkvb: rows = 4 heads x (nope128+v128) -> [nope x4 | v x4]
        Kc = Wkvb_eff[c * NHC * (NOPE + VD):(c + 1) * NHC * (NOPE + VD)]  # [1024, KVLR]
        nk = np.zeros((1024, KVLR), np.float32)
        for h in range(NHC):
            nk[h * P:(h + 1) * P] = Kc[h * (NOPE + VD):h * (NOPE + VD) + NOPE]
            nk[512 + h * P:512 + (h + 1) * P] = \
                Kc[h * (NOPE + VD) + NOPE:(h + 1) * (NOPE + VD)]
        osl = slice(c * NHC * VD, (c + 1) * NHC * VD)
        in_maps.append({
            "hp": hp,
            "wf": wf,
            "wqb": _pack_ktile(np.ascontiguousarray(nw.T), 768),
            "wkvb": _pack_ktile(np.ascontiguousarray(nk.T), 1024),
            "wo": _pack_ktile(np.ascontiguousarray(Wo[:, osl].T), H),
            "cq2": cq2, "sq2": sq2,
            "cqs": np.ascontiguousarray(cos[:, csl]).astype(NPBF),
            "sqs": np.ascontiguousarray(ssin[:, csl]).astype(NPBF),
            "mdg": mdg, "maskT": maskT,
        })

    nc = build(causal)
    trace = bool(os.environ.get("KPROF"))
    res = run_bass_kernel_spmd(nc, in_maps, list(range(NCORES)), trace=trace)
    if trace:
        global LAST_RES
        LAST_RES = res
        print(f"HW exec time: {res.exec_time_ns} ns (mean {res.mean_exec_time_ns}, "
              f"max core {res.max_exec_time_core_id})")
    acc = res.results[0]["outT"].astype(np.float32)
    for c in range(1, NCORES):
        acc += res.results[c]["outT"].astype(np.float32)
    return np.ascontiguousarray(acc.T)[None, :, :].astype(np.float32)
